# revision 37
# baseline (speedup 1.0000x reference)
"""GAT/GCN message-passing layer on 8 Trainium2 NeuronCores.

Math: per query node i the reference computes
    e[i,j] = f_src[i] + f_dst[j]   (masked by Ahat>0), attn = softmax_j, then
    out = relu(attn @ h_prime).
f_src[i] is constant along the softmax axis so it cancels; with g = exp(f_dst)
the layer collapses to one GEMM over the adjacency:
    out = relu( (Ahat @ [g*h' | g])[:, :256] / (Ahat @ [g*h' | g])[:, 256:] )
with h' = node_feats @ w and f_dst = node_feats @ (w @ w_a @ a[2:4]).

Sharding: 1D row partition of query nodes; each core owns 1024 output rows and
streams its [8192, 1024] adjacency slice (transposed so the contraction axis j
lands on SBUF partitions).  w/u/node_feats^T are replicated; every core
recomputes the B = [g*h' | g] panel locally.

Key speed tricks vs. the bf16 baseline:
  * The adjacency is binary, so fp8e4 is lossless for it: half the HBM bytes,
    and the whole 8.4MB/core slice fits pinned in SBUF - no streaming ring,
    no refill choreography.  The main GEMM runs with mixed dtypes: fp8 A as
    the stationary operand, bf16 B moving (the PE allows mixed non-fp32
    inputs), so B keeps full bf16 accuracy (end-to-end rel-err ~2.4e-3).
  * All DRAM tensors are laid out as pre-tiled SBUF images (partition-major),
    so every DMA is 128 contiguous descriptors instead of 1024 - descriptor
    generation on the sequencers drops from ~8us to ~1us per load.
  * The prefix (h' panel) is interleaved with the main GEMM two blocks at a
    time (prefix 2k, 2k+1, then main 2k-4, 2k-3): h' borrows PSUM banks 6/7
    while main i-blocks 0-5 accumulate in banks 0-5, and the two-main-block
    slack keeps the bank-WAR (prefix j+2 vs B-prep's readers of hp[j]) off
    the PE critical path.  i-blocks 6/7 are backfilled bank-major after the
    prefix finishes (all of A is resident, so the backfill is pure PE work,
    and bank 6's epilogue/store overlap bank 7's matmuls).

walrus accepts only ONE sync wait per instruction, so the dataflow keeps each
instruction's cross-engine deps on a single engine: all of B-prep lives on
ACT (exp then scale-copy, so the PE wait of the copy is dominated by exp's
and elided), main matmuls wait only on ACT, DMA first-touches are absorbed by
PE nops at chunk boundaries, and the output stores go through gpsimd whose
DMA queues carry no load traffic.
"""

import sys

import ml_dtypes
import numpy as np

sys.path.insert(0, "/opt/trn_rl_repo")

import concourse.bass as bass  # noqa: E402
import concourse.tile as tile  # noqa: E402
from concourse import mybir  # noqa: E402
from concourse.bass_utils import run_bass_kernel_spmd  # noqa: E402
from concourse.tile import add_dep_helper  # noqa: E402

N = 8192
F = 256  # in_features == out_features
FE = F + 1  # h' columns + the g column
NCORES = 8
ROWS = N // NCORES  # 1024 output rows per core
P = 128
NJ = N // P  # 64 contraction blocks
NI = ROWS // P  # 8 output-row blocks per core

BF = mybir.dt.bfloat16
F8 = mybir.dt.float8e4
F32 = mybir.dt.float32

# params image column split: chunk0 = wext + nfT blocks 0..1 (tiny, so the
# first prefix matmul starts right after the preamble), then 2..7, 8..31, 32..63
PSPLIT = [0, FE + 2 * P, FE + 8 * P, FE + 32 * P, FE + NJ * P]
# adjacency chunk split, in j-blocks: small first chunks so the first main
# j-blocks can start while params stream
ASPLIT = [0, 2, 8, 16, 24, 32, 40, 48, 56, 64]

_CACHE = {}


def _build():
    nc = bass.Bass(
        "TRN2",
        target_bir_lowering=False,
        debug=False,
        enable_asserts=True,
        num_devices=NCORES,
    )
    # pre-tiled images (partition-major; see _prep_inputs)
    aT = nc.dram_tensor("aT", [P, NJ, ROWS], F8, kind="ExternalInput").ap()
    pchunks = tuple(
        nc.dram_tensor(
            f"p{c}", [P, 2, PSPLIT[c + 1] - PSPLIT[c]], BF, kind="ExternalInput"
        ).ap()
        for c in range(4)
    )
    out = nc.dram_tensor("out", [P, NI, F], F32, kind="ExternalOutput").ap()

    with tile.TileContext(nc) as tc:
        _body(tc, aT, pchunks, out)
    return nc


def _body(tc, aT, params, out):
    nc = tc.nc
    Exp = mybir.ActivationFunctionType.Exp
    Relu = mybir.ActivationFunctionType.Relu

    with (
        tc.tile_pool(name="consts", bufs=1) as consts,
        tc.tile_pool(name="rpool", bufs=8) as rpool,
        tc.tile_pool(name="psum", bufs=1, space="PSUM") as psum,
    ):
        # ---- SBUF tiles ----------------------------------------------------
        p_sb = [
            consts.tile(
                [P, 2, PSPLIT[c + 1] - PSPLIT[c]], BF, tag=f"p{c}", name=f"p{c}"
            )
            for c in range(4)
        ]
        aT_sb = consts.tile([P, NJ, ROWS], F8, tag="aT")
        Bp = consts.tile([P, NJ, FE], BF, tag="Bp")  # [g*h' | g] panel
        G = consts.tile([P, NJ], F32, tag="G")  # g = exp(f_dst)
        otile = consts.tile([P, NI * F], F32, tag="o")

        PJLO = [None, 2, 8, 32]

        def nfT(j, kb):
            """SBUF [128, 128] lhsT view of node_feats^T block j, k-half kb."""
            if j < 2:
                return p_sb[0][:, kb, FE + j * P : FE + (j + 1) * P]
            for c in range(3, 0, -1):
                if j >= PJLO[c]:
                    return p_sb[c][:, kb, (j - PJLO[c]) * P : (j - PJLO[c] + 1) * P]

        wext = p_sb[0][:, :, 0:FE]

        # ---- loads ----------------------------------------------------------
        # all loads on SYNC (gpsimd's DMA queues stay virgin for the output
        # stores, which otherwise pick up a queue-reuse wait on top of their
        # data wait); each DMA is 128 contiguous descriptors.
        pdma = []
        prev = None
        for c in range(4):
            d = nc.sync.dma_start(p_sb[c][:], params[c][:])
            if prev is not None:
                add_dep_helper(d.ins, prev.ins, sync=False, reason="pdma order")
            prev = d
            pdma.append(d)
        adma = []
        for c in range(len(ASPLIT) - 1):
            lo, hi = ASPLIT[c], ASPLIT[c + 1]
            d = nc.sync.dma_start(aT_sb[:, lo:hi, :], aT[:, lo:hi, :])
            add_dep_helper(d.ins, prev.ins, sync=False, reason="adma order")
            prev = d
            adma.append(d)
        # keep the params ahead of the adjacency bulk on the HBM fabric
        add_dep_helper(adma[2].ins, pdma[3].ins, reason="params first")

        # ---- PSUM accumulators ----------------------------------------------
        acc = [
            psum.tile([P, FE], F32, tag=f"acc{i}", name=f"acc{i}") for i in range(NI)
        ]
        # prefix h' borrows banks 6/7 (i-blocks 6/7 are backfilled later);
        # blocks 0..5 fan out across the still-idle main banks so the early
        # bank-WAR chain (prefix j+2 vs B-prep readers of hp[j]) vanishes
        # while the pipeline fills
        def hp_bank(j):
            if j < 6:
                return acc[[6, 7, 0, 1, 2, 3][j]]
            return acc[6 + j % 2]

        # first-touch absorbers: a PE nop waits on the DMA so the matmul that
        # follows needs only its ACT wait
        def pe_gate(dma):
            nop = nc.tensor.nop(nofuse=True, hint="dma_gate")
            add_dep_helper(nop.ins, dma.ins, reason="dma gate")
            return nop


        prev_act = None
        prev_dve = None
        last_mm = None

        def prefix(j):
            nonlocal prev_act, prev_dve, last_mm
            h = hp_bank(j)
            for kb in range(2):
                last_mm = nc.tensor.matmul(
                    h[:],
                    lhsT=nfT(j, kb),
                    rhs=wext[:, kb, :],
                    start=(kb == 0),
                    stop=(kb == 1),
                )
            # G[:, j] = exp(f_dst); takes the single PE wait for this bank so
            # the Bp write below needs none (dominated, elided)
            ex = nc.scalar.activation(G[:, j : j + 1], h[:, F : F + 1], Exp)
            if prev_act is not None:
                add_dep_helper(ex.ins, prev_act.ins, sync=False, reason="act order")
            # Bp[j] = bf16(h' * g)
            bp = nc.scalar.mul(Bp[:, j, 0:F], h[:, 0:F], G[:, j : j + 1])
            add_dep_helper(bp.ins, ex.ins, sync=False, reason="act order")
            prev_act = bp
            if j % 4 == 3:
                # drop this 4-group's g columns into Bp (one strided cast-copy)
                j0 = j - 3
                gh = nc.scalar.copy(Bp[:, j0 : j + 1, F], G[:, j0 : j + 1])
                add_dep_helper(gh.ins, prev_act.ins, sync=False, reason="act order")
                prev_act = gh

        def main_block(j, ilist, start, stop):
            nonlocal last_mm
            for i in ilist:
                last_mm = nc.tensor.matmul(
                    acc[i][:],
                    lhsT=aT_sb[:, j, i * P : (i + 1) * P],
                    rhs=Bp[:, j, :],
                    start=start,
                    stop=stop,
                )

        # ---- interleaved prefix + main stream --------------------------------
        # adjacency chunk c covers j-blocks ASPLIT[c] .. ASPLIT[c+1]-1
        a_gate = {ASPLIT[c]: adma[c] for c in range(len(adma))}
        p_gate = {2: pdma[1], 8: pdma[2], 32: pdma[3]}
        pe_gate(pdma[0])
        for k in range(NJ // 2):
            for j in (2 * k, 2 * k + 1):
                if j in p_gate:
                    pe_gate(p_gate[j])
                prefix(j)
            for jm in (2 * k - 4, 2 * k - 3):
                if jm < 0:
                    continue
                if jm in a_gate:
                    pe_gate(a_gate[jm])
                main_block(jm, range(6), start=(jm == 0), stop=False)
        for jm in range(NJ - 4, NJ):
            main_block(jm, range(6), start=False, stop=(jm == NJ - 1))
        # backfill i-blocks 6/7 (banks 6/7 are free once the prefix drained);
        # bank-major so bank 6's epilogue and store overlap bank 7's matmuls
        for i in (6, 7):
            for j in range(NJ):
                main_block(j, (i,), start=(j == 0), stop=(j == NJ - 1))

        # ---- epilogue: out[i] = relu(acc[i][:, :F] / acc[i][:, F]) ----------
        # banks 0..5 finish at the end of the interleaved stream and drain on
        # ACT while the backfill matmuls still run; banks 6/7 drain via DVE.
        stores = []
        banksA = list(range(6))
        denomA = rpool.tile([P, len(banksA)], F32, tag="denomA")
        denom_last = None
        for k, i in enumerate(banksA):
            dc = nc.scalar.copy(denomA[:, k : k + 1], acc[i][:, F : F + 1])
            if denom_last is not None:
                add_dep_helper(dc.ins, denom_last.ins, sync=False, reason="act order")
            denom_last = dc
        recipA = rpool.tile([P, len(banksA)], F32, tag="recipA")
        nc.vector.reciprocal(recipA[:], denomA[:])
        # sacrificial ACT read absorbs the DVE tick for the six fused relus
        sacA = rpool.tile([P, len(banksA)], F32, tag="sacA")
        sa = nc.scalar.copy(sacA[:], recipA[:])
        add_dep_helper(sa.ins, denom_last.ins, sync=False, reason="act order")
        last_relu = sa
        for k, i in enumerate(banksA):
            o = otile[:, i * F : (i + 1) * F]
            rl = nc.scalar.activation(
                o, acc[i][:, 0:F], Relu, scale=recipA[:, k : k + 1]
            )
            add_dep_helper(rl.ins, last_relu.ins, sync=False, reason="act order")
            last_relu = rl
        # gpsimd nop absorbs the ACT dep so the store itself carries only its
        # DMA-queue wait
        gnop = nc.gpsimd.nop(nofuse=True, hint="storeA_gate")
        add_dep_helper(gnop.ins, last_relu.ins, reason="storeA gate")
        stores.append(nc.gpsimd.dma_start(out[:, 0:6, :], otile[:, 0 : 6 * F]))
        add_dep_helper(stores[-1].ins, gnop.ins, sync=False, reason="after gate")

        # banks 6/7 (backfill): DVE path, one chain per bank so bank 6's
        # store overlaps bank 7's backfill matmuls
        denomB = rpool.tile([P, 2], F32, tag="denomB")
        recipB = rpool.tile([P, 2], F32, tag="recipB")
        last_dve = None
        for k, i in enumerate([6, 7]):
            # DVE-only chain: the 1-col denominator copy takes the PE wait
            nc.vector.tensor_copy(denomB[:, k : k + 1], acc[i][:, F : F + 1])
            nc.vector.reciprocal(recipB[:, k : k + 1], denomB[:, k : k + 1])
            o = otile[:, i * F : (i + 1) * F]
            nc.vector.tensor_scalar_mul(o, acc[i][:, 0:F], recipB[:, k : k + 1])
            last_dve = nc.vector.tensor_scalar_max(o, o, 0.0)
            gnop = nc.gpsimd.nop(nofuse=True, hint=f"store{i}_gate")
            add_dep_helper(gnop.ins, last_dve.ins, reason=f"store{i} gate")
            stores.append(
                nc.gpsimd.dma_start(out[:, i : i + 1, :], otile[:, i * F : (i + 1) * F])
            )
            add_dep_helper(stores[-1].ins, gnop.ins, sync=False, reason="after gate")

        # funnel every proc's final tick into SP via single-wait nops so the
        # kernel-tail drain has nothing left to wait on (every DMA queue's
        # final count included, else the drain aggregates 10+ waits)
        for dep in [*pdma, *adma, *stores, last_mm, last_relu, last_dve, prev_act]:
            nop = nc.sync.nop(nofuse=True, hint="tail_funnel")
            add_dep_helper(nop.ins, dep.ins, reason="tail funnel")


def _prep_inputs(node_feats, Ahat, w, w_a, a):
    node_feats = np.asarray(node_feats, dtype=np.float32)
    Ahat = np.asarray(Ahat, dtype=np.float32)
    w = np.asarray(w, dtype=np.float32)
    w_a = np.asarray(w_a, dtype=np.float32)
    a = np.asarray(a, dtype=np.float32)

    u = w @ (w_a @ a[2:4])  # [256, 1]
    # params matrix [256, 257 + 8192] -> partition-major image [128, 2, *]
    M = np.concatenate([w, u, node_feats.T], axis=1).astype(ml_dtypes.bfloat16)
    img = M.reshape(2, P, -1).transpose(1, 0, 2)
    pchunks = {
        f"p{c}": np.ascontiguousarray(img[:, :, PSPLIT[c] : PSPLIT[c + 1]])
        for c in range(4)
    }

    in_maps = []
    for c in range(NCORES):
        aT_c = Ahat[c * ROWS : (c + 1) * ROWS, :].T  # [8192, 1024]
        aT_img = np.ascontiguousarray(
            aT_c.reshape(NJ, P, ROWS).transpose(1, 0, 2).astype(ml_dtypes.float8_e4m3)
        )
        in_maps.append({"aT": aT_img, **pchunks})
    return in_maps


def _run(inputs, trace=False, **kwargs):
    if "nc" not in _CACHE:
        _CACHE["nc"] = _build()
    nc = _CACHE["nc"]
    in_maps = _prep_inputs(**inputs)
    res = run_bass_kernel_spmd(
        nc, in_maps, core_ids=list(range(NCORES)), trace=trace, **kwargs
    )
    # out image [128, 8, 256] -> rows (i*128 + p)
    full = np.concatenate(
        [
            res.results[c]["out"].transpose(1, 0, 2).reshape(ROWS, F)
            for c in range(NCORES)
        ],
        axis=0,
    )
    return full, res


def kernel(**inputs) -> np.ndarray:
    out, _ = _run(inputs, trace=False)
    return out
